# revision 6
# baseline (speedup 1.0000x reference)
"""KQEnergyBlock Trainium2 Bass kernel.

Math (per batch element b, all derived from the reference):
  Q = x @ Wq^T, K = x @ Wk^T                      (N, D), heads h: slices of 64
  S_h = beta_h * Q_h @ K_h^T                      (N, N)
  A_h = softmax(S_h, axis=-1) = E_h / r_h,  E_h = exp(S_h), r_h = rowsum(E_h)
  T1  = sum_h (A_h @ K_h) @ Wq_r[h]  = AVc  @ Wq   (AVc  = concat_h A_h @ K_h)
  T2  = sum_h (A_h^T @ Q_h) @ Wk_r[h] = ATQc @ Wk  (ATQc = concat_h A_h^T @ Q_h)
  mlp = relu(x @ Wm^T) @ Wm
  out = T1 + T2 + mlp

Sharding: data-parallel over batch B=8 across the 8 NeuronCores, one batch
element per core; weights replicated. No collectives.

On-chip layouts (partition dim first):
  xT   [128, 6, 1024]  bf16   (d = c*128+p, n)
  QT   [128, 6, 1024]  bf16   beta-scaled Q^T  (e = c*128+p, q)
  KT   [128, 6, 1024]  bf16   K^T
  Qn   [128, 8,  768]  bf16   Q natural (q = no*128+p, e), raw
  Kn   [128, 8,  768]  bf16   K natural
  E    [128, 8, 1024]  bf16   exp(S_h)   (q = qo*128+p, k)
  ET   [128, 8, 1024]  bf16   exp(S_h^T) (k = ko*128+p, q)
  AVT  [128, 6, 1024]  bf16   concat_h (A_h @ K_h)^T   (e, q)
  ATQT [128, 6, 1024]  bf16   concat_h (A_h^T @ Q_h)^T (e, k)
  hid spilled to DRAM as [24, 128, 1024] bf16 (hid = ho*128+p, n)
"""

import numpy as np
import ml_dtypes

import concourse.mybir as mybir
import concourse.tile as tile
from concourse import bacc
from concourse.bass_utils import run_bass_kernel_spmd

B, N, D = 8, 1024, 768
H, Z = 12, 64
HID = 3072
P = 128
DC = D // P     # 6
NC = N // P     # 8
HC = HID // P   # 24
BF = mybir.dt.bfloat16
F32 = mybir.dt.float32
Exp = mybir.ActivationFunctionType.Exp
Relu = mybir.ActivationFunctionType.Relu
Copy = mybir.ActivationFunctionType.Copy
Mult = mybir.AluOpType.mult

NPBF = ml_dtypes.bfloat16

_CACHE = {}


def _build():
    nc = bacc.Bacc("TRN2", target_bir_lowering=False, debug=False, num_devices=8)
    xT_d = nc.dram_tensor("xT", [D, N], BF, kind="ExternalInput")
    wqT_d = nc.dram_tensor("wqT", [D, D], BF, kind="ExternalInput")
    wkT_d = nc.dram_tensor("wkT", [D, D], BF, kind="ExternalInput")
    wq_d = nc.dram_tensor("wq", [D, D], BF, kind="ExternalInput")
    wk_d = nc.dram_tensor("wk", [D, D], BF, kind="ExternalInput")
    wmT_d = nc.dram_tensor("wmT", [D, HID], BF, kind="ExternalInput")
    wm_d = nc.dram_tensor("wm", [HID, D], BF, kind="ExternalInput")
    qscale_d = nc.dram_tensor("qscale", [P, DC], F32, kind="ExternalInput")
    out_d = nc.dram_tensor("out", [N, D], F32, kind="ExternalOutput")

    # DRAM views with the partition dim innermost-of-row-chunk
    xT_v = xT_d.ap().rearrange("(c p) n -> p c n", p=P)      # [128, 6, 1024]
    wqT_v = wqT_d.ap().rearrange("(c p) e -> p c e", p=P)    # [128, 6, 768]
    wkT_v = wkT_d.ap().rearrange("(c p) e -> p c e", p=P)
    wq_v = wq_d.ap().rearrange("(c p) d -> p c d", p=P)
    wk_v = wk_d.ap().rearrange("(c p) d -> p c d", p=P)
    wmT_v = wmT_d.ap().rearrange("(c p) h -> p c h", p=P)    # [128, 6, 3072]
    wm_v = wm_d.ap().rearrange("(c p) d -> p c d", p=P)      # [128, 24, 768]
    out_v = out_d.ap().rearrange("(c p) d -> p c d", p=P)    # [128, 8, 768]

    with tile.TileContext(nc) as tc:
        with (
            tc.tile_pool(name="acts", bufs=1) as acts,
            tc.tile_pool(name="hd", bufs=1) as hd,
            tc.tile_pool(name="stream", bufs=3) as stream,
            tc.tile_pool(name="ps", bufs=3, space="PSUM") as ps,
            tc.tile_pool(name="ps2", bufs=2, space="PSUM") as ps2,
            tc.tile_pool(name="dram", bufs=1, space="DRAM") as dram,
        ):
            # ---- input loads ----
            xT = acts.tile([P, DC, N], BF)
            wqT = acts.tile([P, DC, D], BF)
            wkT = acts.tile([P, DC, D], BF)
            wq = acts.tile([P, DC, D], BF)
            wk = acts.tile([P, DC, D], BF)
            qscale = acts.tile([P, DC], F32)
            nc.sync.dma_start(xT[:], xT_v)
            nc.sync.dma_start(wqT[:], wqT_v)
            nc.sync.dma_start(wkT[:], wkT_v)
            nc.sync.dma_start(wq[:], wq_v)
            nc.sync.dma_start(wk[:], wk_v)
            nc.sync.dma_start(qscale[:], qscale_d.ap())

            QT = acts.tile([P, DC, N], BF)
            KT = acts.tile([P, DC, N], BF)
            Qn = acts.tile([P, NC, D], BF)
            Kn = acts.tile([P, NC, D], BF)
            AVT = acts.tile([P, DC, N], BF)
            ATQT = acts.tile([P, DC, N], BF)
            hid_dram = dram.tile([HC, P, N], BF)

            # ---- stage 1: projections ----
            # QT/KT (feature-major):  psum[e_chunk, n] += wT[:, do, e_chunk].T @ xT[:, do, nh]
            for wT_sb, dst, scaled in ((wqT, QT, True), (wkT, KT, False)):
                for eo in range(DC):
                    pt = ps.tile([P, N], F32, tag="ps_big", name="pt")
                    for do in range(DC):
                        for nh in range(2):
                            nc.tensor.matmul(
                                pt[:, nh * 512:(nh + 1) * 512],
                                wT_sb[:, do, eo * P:(eo + 1) * P],
                                xT[:, do, nh * 512:(nh + 1) * 512],
                                start=(do == 0), stop=(do == DC - 1),
                            )
                    if scaled:
                        nc.scalar.activation(dst[:, eo, :], pt[:], Copy,
                                             scale=qscale[:, eo:eo + 1])
                    else:
                        nc.scalar.copy(dst[:, eo, :], pt[:])
            # Qn/Kn (natural): psum[n_chunk, e] += xT[:, do, n_chunk].T @ wT[:, do, eslice]
            for wT_sb, dst in ((wqT, Qn), (wkT, Kn)):
                for no in range(NC):
                    pt = ps.tile([P, N], F32, tag="ps_big", name="pt")
                    for do in range(DC):
                        nc.tensor.matmul(
                            pt[:, 0:512],
                            xT[:, do, no * P:(no + 1) * P],
                            wT_sb[:, do, 0:512],
                            start=(do == 0), stop=(do == DC - 1),
                        )
                        nc.tensor.matmul(
                            pt[:, 512:768],
                            xT[:, do, no * P:(no + 1) * P],
                            wT_sb[:, do, 512:768],
                            start=(do == 0), stop=(do == DC - 1),
                        )
                    nc.vector.tensor_copy(dst[:, no, :], pt[:, 0:768])

            # ---- stage 2: MLP layer 1 (hid spilled to DRAM) ----
            for ho in range(HC):
                wt = stream.tile([P, DC, P], BF, tag="wmT", name="wt")
                nc.sync.dma_start(wt[:], wmT_v[:, :, ho * P:(ho + 1) * P])
                pt = ps.tile([P, N], F32, tag="ps_big", name="pt")
                for do in range(DC):
                    for nh in range(2):
                        nc.tensor.matmul(
                            pt[:, nh * 512:(nh + 1) * 512],
                            wt[:, do, :],
                            xT[:, do, nh * 512:(nh + 1) * 512],
                            start=(do == 0), stop=(do == DC - 1),
                        )
                hchunk = stream.tile([P, N], BF, tag="hchunk", name="hchunk")
                nc.scalar.activation(hchunk[:], pt[:], Relu)
                nc.sync.dma_start(hid_dram[ho], hchunk[:])

            # ---- stage 3: per-head attention ----
            for h in range(H):
                zo = (h % 2) * Z
                c = h // 2
                QT_h = QT[zo:zo + Z, c, :]   # (64, 1024) z x q, beta-scaled
                KT_h = KT[zo:zo + Z, c, :]   # (64, 1024) z x k

                # S (q x k) -> E = exp(S), rowsum in r_col
                E = hd.tile([P, NC, N], BF, tag="E", name="E")
                r_col = hd.tile([P, NC], F32, tag="r_col", name="r_col", bufs=2)
                for qo in range(NC):
                    pt = ps.tile([P, N], F32, tag="ps_big", name="pt")
                    for kh in range(2):
                        nc.tensor.matmul(
                            pt[:, kh * 512:(kh + 1) * 512],
                            QT_h[:, qo * P:(qo + 1) * P],
                            KT_h[:, kh * 512:(kh + 1) * 512],
                            start=True, stop=True,
                        )
                    nc.scalar.activation(E[:, qo, :], pt[:], Exp,
                                         accum_out=r_col[:, qo:qo + 1])
                # S^T (k x q) -> ET
                ET = hd.tile([P, NC, N], BF, tag="ET", name="ET")
                for ko in range(NC):
                    pt = ps.tile([P, N], F32, tag="ps_big", name="pt")
                    for qh in range(2):
                        nc.tensor.matmul(
                            pt[:, qh * 512:(qh + 1) * 512],
                            KT_h[:, ko * P:(ko + 1) * P],
                            QT_h[:, qh * 512:(qh + 1) * 512],
                            start=True, stop=True,
                        )
                    nc.scalar.activation(ET[:, ko, :], pt[:], Exp)

                # K natural slice for this head + ones column (for rowsum row)
                Kn1 = hd.tile([P, NC, Z + 1], BF, tag="Kn1", name="Kn1", bufs=2)
                for ko in range(NC):
                    nc.vector.tensor_copy(Kn1[:, ko, 0:Z], Kn[:, ko, h * Z:(h + 1) * Z])
                nc.vector.memset(Kn1[:, :, Z:Z + 1], 1.0)

                # AV^T (z x q) with appended rowsum row; normalize by 1/r_row
                for qh in range(2):
                    rr_inv = hd.tile([1, 512], F32, tag="rr_inv", name="rr_inv", bufs=2)
                    pa = ps2.tile([P, 512], F32, tag="ps_med", name="pa")
                    for ko in range(NC):
                        nc.tensor.matmul(
                            pa[0:Z + 1, :],
                            Kn1[:, ko, :],
                            ET[:, ko, qh * 512:(qh + 1) * 512],
                            start=(ko == 0), stop=(ko == NC - 1),
                        )
                    nc.vector.reciprocal(rr_inv[:], pa[Z:Z + 1, :])
                    rr_bc = hd.tile([Z, 512], F32, tag="rr_bc", name="rr_bc", bufs=2)
                    nc.gpsimd.partition_broadcast(rr_bc[:], rr_inv[0:1, :])
                    nc.vector.tensor_tensor(
                        AVT[zo:zo + Z, c, qh * 512:(qh + 1) * 512],
                        pa[0:Z, :],
                        rr_bc[:],
                        Mult,
                    )

                # Qr = Q_nat * (1/r_col) rowwise
                rc_inv = hd.tile([P, NC], F32, tag="rc_inv", name="rc_inv", bufs=2)
                nc.vector.reciprocal(rc_inv[:], r_col[:])
                Qr = hd.tile([P, NC, Z], BF, tag="Qr", name="Qr", bufs=2)
                for qo in range(NC):
                    nc.vector.tensor_scalar_mul(
                        Qr[:, qo, :], Qn[:, qo, h * Z:(h + 1) * Z],
                        rc_inv[:, qo:qo + 1],
                    )

                # ATQ^T (z x k) = Qr^T-contracted with E
                for kh in range(2):
                    pa = ps2.tile([P, 512], F32, tag="ps_med", name="pa")
                    for qo in range(NC):
                        nc.tensor.matmul(
                            pa[0:Z, :],
                            Qr[:, qo, :],
                            E[:, qo, kh * 512:(kh + 1) * 512],
                            start=(qo == 0), stop=(qo == NC - 1),
                        )
                    nc.vector.tensor_copy(
                        ATQT[zo:zo + Z, c, kh * 512:(kh + 1) * 512], pa[0:Z, :])

            # ---- stage 4: out = AVc @ Wq + ATQc @ Wk + hid @ Wm ----
            for nos in ([0, 1, 2], [3, 4, 5], [6, 7]):   # groups of <=3 n-chunks
                pouts = []
                for i in range(len(nos)):
                    po = ps.tile([P, N], F32, tag="ps_big", name="po")
                    pouts.append(po)
                for i, no in enumerate(nos):
                    for c2 in range(DC):
                        for lhs, w_sb in ((AVT, wq), (ATQT, wk)):
                            nc.tensor.matmul(
                                pouts[i][:, 0:512],
                                lhs[:, c2, no * P:(no + 1) * P],
                                w_sb[:, c2, 0:512],
                                start=(c2 == 0 and lhs is AVT), stop=False,
                            )
                            nc.tensor.matmul(
                                pouts[i][:, 512:768],
                                lhs[:, c2, no * P:(no + 1) * P],
                                w_sb[:, c2, 512:768],
                                start=(c2 == 0 and lhs is AVT), stop=False,
                            )
                for ho in range(HC):
                    wmc = stream.tile([P, D], BF, tag="wmc", name="wmc")
                    nc.sync.dma_start(wmc[:], wm_v[:, ho, :])
                    hc = stream.tile([P, N], BF, tag="hc", name="hc")
                    nc.sync.dma_start(hc[:], hid_dram[ho])
                    for i, no in enumerate(nos):
                        nc.tensor.matmul(
                            pouts[i][:, 0:512],
                            hc[:, no * P:(no + 1) * P],
                            wmc[:, 0:512],
                            start=False, stop=(ho == HC - 1),
                        )
                        nc.tensor.matmul(
                            pouts[i][:, 512:768],
                            hc[:, no * P:(no + 1) * P],
                            wmc[:, 512:768],
                            start=False, stop=(ho == HC - 1),
                        )
                for i, no in enumerate(nos):
                    osb = stream.tile([P, D], F32, tag="osb", name="osb", bufs=2)
                    nc.vector.tensor_copy(osb[:], pouts[i][:, 0:768])
                    nc.sync.dma_start(out_v[:, no, :], osb[:])

    nc.compile()
    return nc


def _prep(x, Wq, Wk, betas, W_mlp):
    x = np.asarray(x, dtype=np.float32)
    Wq = np.asarray(Wq, dtype=np.float32)
    Wk = np.asarray(Wk, dtype=np.float32)
    betas = np.asarray(betas, dtype=np.float32)
    W_mlp = np.asarray(W_mlp, dtype=np.float32)

    wq = np.ascontiguousarray(Wq).astype(NPBF)
    wk = np.ascontiguousarray(Wk).astype(NPBF)
    wqT = np.ascontiguousarray(Wq.T).astype(NPBF)
    wkT = np.ascontiguousarray(Wk.T).astype(NPBF)
    wm = np.ascontiguousarray(W_mlp).astype(NPBF)
    wmT = np.ascontiguousarray(W_mlp.T).astype(NPBF)
    # qscale[p, c] = betas[(c*128+p)//64]
    e_idx = (np.arange(DC)[None, :] * P + np.arange(P)[:, None]) // Z
    qscale = betas[e_idx].astype(np.float32)

    in_maps = []
    for b in range(B):
        xT = np.ascontiguousarray(x[b].T).astype(NPBF)
        in_maps.append({
            "xT": xT, "wqT": wqT, "wkT": wkT, "wq": wq, "wk": wk,
            "wmT": wmT, "wm": wm, "qscale": qscale,
        })
    return in_maps


def kernel(x, Wq, Wk, betas, W_mlp, _trace=False):
    if "nc" not in _CACHE:
        _CACHE["nc"] = _build()
    nc = _CACHE["nc"]
    in_maps = _prep(x, Wq, Wk, betas, W_mlp)
    res = run_bass_kernel_spmd(nc, in_maps, core_ids=list(range(B)), trace=_trace)
    out = np.stack([res.results[b]["out"] for b in range(B)], axis=0)
    _CACHE["last_result"] = res
    return out.astype(np.float32)


# revision 11
# speedup vs baseline: 7446.6667x; 7446.6667x over previous
"""KQEnergyBlock Trainium2 Bass kernel.

Math (per batch element b, all derived from the reference):
  Q = x @ Wq^T, K = x @ Wk^T                      (N, D), heads h: slices of 64
  S_h = beta_h * Q_h @ K_h^T                      (N, N)
  A_h = softmax(S_h, axis=-1) = E_h / r_h,  E_h = exp(S_h), r_h = rowsum(E_h)
  T1  = sum_h (A_h @ K_h) @ Wq_r[h]  = AVc  @ Wq   (AVc  = concat_h A_h @ K_h)
  T2  = sum_h (A_h^T @ Q_h) @ Wk_r[h] = ATQc @ Wk  (ATQc = concat_h A_h^T @ Q_h)
  mlp = relu(x @ Wm^T) @ Wm
  out = T1 + T2 + mlp

Sharding: data-parallel over batch B=8 across the 8 NeuronCores, one batch
element per core; weights replicated. No collectives.

On-chip layouts (partition dim first):
  xT   [128, 6, 1024]  bf16   (d = c*128+p, n)
  QT   [128, 6, 1024]  bf16   beta-scaled Q^T  (e = c*128+p, q)
  KT   [128, 6, 1024]  bf16   K^T
  Qn   [128, 8,  768]  bf16   Q natural (q = no*128+p, e), raw
  Kn   [128, 8,  768]  bf16   K natural
  E    [128, 8, 1024]  bf16   exp(S_h)   (q = qo*128+p, k)
  ET   [128, 8, 1024]  bf16   exp(S_h^T) (k = ko*128+p, q)
  AVT  [128, 6, 1024]  bf16   concat_h (A_h @ K_h)^T   (e, q)
  ATQT [128, 6, 1024]  bf16   concat_h (A_h^T @ Q_h)^T (e, k)
  hid spilled to DRAM as [24, 128, 1024] bf16 (hid = ho*128+p, n)
"""

import numpy as np
import ml_dtypes

import concourse.mybir as mybir
import concourse.tile as tile
from concourse import bacc
from concourse.bass_utils import run_bass_kernel_spmd

B, N, D = 8, 1024, 768
H, Z = 12, 64
HID = 3072
P = 128
DC = D // P     # 6
NC = N // P     # 8
HC = HID // P   # 24
BF = mybir.dt.bfloat16
F32 = mybir.dt.float32
Exp = mybir.ActivationFunctionType.Exp
Relu = mybir.ActivationFunctionType.Relu
Copy = mybir.ActivationFunctionType.Copy
Mult = mybir.AluOpType.mult

NPBF = ml_dtypes.bfloat16

_CACHE = {}


def _build():
    nc = bacc.Bacc("TRN2", target_bir_lowering=False, debug=False, num_devices=8)
    xT_d = nc.dram_tensor("xT", [D, N], BF, kind="ExternalInput")
    wqT_d = nc.dram_tensor("wqT", [D, D], BF, kind="ExternalInput")
    wkT_d = nc.dram_tensor("wkT", [D, D], BF, kind="ExternalInput")
    wq_d = nc.dram_tensor("wq", [D, D], BF, kind="ExternalInput")
    wk_d = nc.dram_tensor("wk", [D, D], BF, kind="ExternalInput")
    wmT_d = nc.dram_tensor("wmT", [D, HID], BF, kind="ExternalInput")
    wm_d = nc.dram_tensor("wm", [HID, D], BF, kind="ExternalInput")
    qscale_d = nc.dram_tensor("qscale", [P, DC], F32, kind="ExternalInput")
    out_d = nc.dram_tensor("out", [N, D], F32, kind="ExternalOutput")

    # DRAM views with the partition dim innermost-of-row-chunk
    xT_v = xT_d.ap().rearrange("(c p) n -> p c n", p=P)      # [128, 6, 1024]
    wqT_v = wqT_d.ap().rearrange("(c p) e -> p c e", p=P)    # [128, 6, 768]
    wkT_v = wkT_d.ap().rearrange("(c p) e -> p c e", p=P)
    wq_v = wq_d.ap().rearrange("(c p) d -> p c d", p=P)
    wk_v = wk_d.ap().rearrange("(c p) d -> p c d", p=P)
    wmT_v = wmT_d.ap().rearrange("(c p) h -> p c h", p=P)    # [128, 6, 3072]
    wm_v = wm_d.ap().rearrange("(c p) d -> p c d", p=P)      # [128, 24, 768]
    out_v = out_d.ap().rearrange("(c p) d -> p c d", p=P)    # [128, 8, 768]

    with tile.TileContext(nc) as tc:
        with (
            tc.tile_pool(name="acts", bufs=1) as acts,
            tc.tile_pool(name="hd", bufs=1) as hd,
            tc.tile_pool(name="stream", bufs=3) as stream,
            tc.tile_pool(name="ps", bufs=4, space="PSUM") as ps,
            tc.tile_pool(name="dram", bufs=1, space="DRAM") as dram,
        ):
            # ---- input loads ----
            xT = acts.tile([P, DC, N], BF)
            wqT = acts.tile([P, DC, D], BF)
            wkT = acts.tile([P, DC, D], BF)
            wq = acts.tile([P, DC, D], BF)
            wk = acts.tile([P, DC, D], BF)
            qscale = acts.tile([P, DC], F32)
            nc.sync.dma_start(xT[:], xT_v)
            nc.sync.dma_start(wqT[:], wqT_v)
            nc.sync.dma_start(wkT[:], wkT_v)
            nc.sync.dma_start(wq[:], wq_v)
            nc.sync.dma_start(wk[:], wk_v)
            nc.sync.dma_start(qscale[:], qscale_d.ap())

            QT = acts.tile([P, DC, N], BF)
            KT = acts.tile([P, DC, N], BF)
            Qn = acts.tile([P, NC, D], BF)
            Kn = acts.tile([P, NC, D], BF)
            AVT = acts.tile([P, DC, N], BF)
            ATQT = acts.tile([P, DC, N], BF)
            hid_dram = dram.tile([HC, P, N], BF)

            # ---- stage 1: projections ----
            # QT/KT (feature-major):  psum[e_chunk, n] += wT[:, do, e_chunk].T @ xT[:, do, nh]
            for wT_sb, dst, scaled in ((wqT, QT, True), (wkT, KT, False)):
                for eo in range(DC):
                    pt = ps.tile([P, N], F32, tag="ps_big", name="pt")
                    for do in range(DC):
                        for nh in range(2):
                            nc.tensor.matmul(
                                pt[:, nh * 512:(nh + 1) * 512],
                                wT_sb[:, do, eo * P:(eo + 1) * P],
                                xT[:, do, nh * 512:(nh + 1) * 512],
                                start=(do == 0), stop=(do == DC - 1),
                            )
                    if scaled:
                        nc.vector.tensor_scalar_mul(dst[:, eo, :], pt[:],
                                                    qscale[:, eo:eo + 1])
                    else:
                        nc.vector.tensor_copy(dst[:, eo, :], pt[:])
            # Qn/Kn (natural): psum[n_chunk, e] += xT[:, do, n_chunk].T @ wT[:, do, eslice]
            for wT_sb, dst in ((wqT, Qn), (wkT, Kn)):
                for no in range(NC):
                    pt = ps.tile([P, N], F32, tag="ps_big", name="pt")
                    for do in range(DC):
                        nc.tensor.matmul(
                            pt[:, 0:512],
                            xT[:, do, no * P:(no + 1) * P],
                            wT_sb[:, do, 0:512],
                            start=(do == 0), stop=(do == DC - 1),
                        )
                        nc.tensor.matmul(
                            pt[:, 512:768],
                            xT[:, do, no * P:(no + 1) * P],
                            wT_sb[:, do, 512:768],
                            start=(do == 0), stop=(do == DC - 1),
                        )
                    nc.vector.tensor_copy(dst[:, no, :], pt[:, 0:768])

            # ---- stage 2: MLP layer 1 (hid spilled to DRAM) ----
            # emitted interleaved with the head loop below to fill PE idle
            # while ACT is busy with exp
            def mlp1_chunk(ho):
                wt = stream.tile([P, DC, P], BF, tag="wmT", name="wt")
                nc.sync.dma_start(wt[:], wmT_v[:, :, ho * P:(ho + 1) * P])
                pt = ps.tile([P, N], F32, tag="ps_big", name="pt")
                for do in range(DC):
                    for nh in range(2):
                        nc.tensor.matmul(
                            pt[:, nh * 512:(nh + 1) * 512],
                            wt[:, do, :],
                            xT[:, do, nh * 512:(nh + 1) * 512],
                            start=(do == 0), stop=(do == DC - 1),
                        )
                hchunk = stream.tile([P, N], BF, tag="hchunk", name="hchunk")
                nc.vector.tensor_scalar_max(hchunk[:], pt[:], 0.0)
                nc.sync.dma_start(hid_dram[ho], hchunk[:])

            # ---- stage 3: per-head attention ----
            for h in range(H):
                zo = (h % 2) * Z
                c = h // 2
                QT_h = QT[zo:zo + Z, c, :]   # (64, 1024) z x q, beta-scaled
                KT_h = KT[zo:zo + Z, c, :]   # (64, 1024) z x k

                # S (q x k) -> E = exp(S), rowsum in r_col
                E = hd.tile([P, NC, N], BF, tag="E", name="E", bufs=2)
                r_col = hd.tile([P, NC], F32, tag="r_col", name="r_col", bufs=2)
                for qo in range(NC):
                    pt = ps.tile([P, N], F32, tag="ps_big", name="pt")
                    for kh in range(2):
                        nc.tensor.matmul(
                            pt[:, kh * 512:(kh + 1) * 512],
                            QT_h[:, qo * P:(qo + 1) * P],
                            KT_h[:, kh * 512:(kh + 1) * 512],
                            start=True, stop=True,
                        )
                    nc.scalar.activation(E[:, qo, :], pt[:], Exp,
                                         accum_out=r_col[:, qo:qo + 1])
                # S^T (k x q) -> ET
                ET = hd.tile([P, NC, N], BF, tag="ET", name="ET")
                for ko in range(NC):
                    pt = ps.tile([P, N], F32, tag="ps_big", name="pt")
                    for qh in range(2):
                        nc.tensor.matmul(
                            pt[:, qh * 512:(qh + 1) * 512],
                            KT_h[:, ko * P:(ko + 1) * P],
                            QT_h[:, qh * 512:(qh + 1) * 512],
                            start=True, stop=True,
                        )
                    nc.scalar.activation(ET[:, ko, :], pt[:], Exp)

                # K natural slice for this head + ones column (for rowsum row)
                Kn1 = hd.tile([P, NC, Z + 1], BF, tag="Kn1", name="Kn1", bufs=2)
                for ko in range(NC):
                    nc.vector.tensor_copy(Kn1[:, ko, 0:Z], Kn[:, ko, h * Z:(h + 1) * Z])
                nc.vector.memset(Kn1[:, :, Z:Z + 1], 1.0)

                # AV^T (z x q) with appended rowsum row; normalize by 1/r_row
                for qh in range(2):
                    rr_inv = hd.tile([1, 512], F32, tag="rr_inv", name="rr_inv", bufs=2)
                    pab = ps.tile([P, N], F32, tag="ps_big", name="pab")
                    pa = pab[:, 0:512]
                    for ko in range(NC):
                        nc.tensor.matmul(
                            pa[0:Z + 1, :],
                            Kn1[:, ko, :],
                            ET[:, ko, qh * 512:(qh + 1) * 512],
                            start=(ko == 0), stop=(ko == NC - 1),
                        )
                    nc.vector.reciprocal(rr_inv[:], pa[Z:Z + 1, :])
                    rr_bc = hd.tile([Z, 512], F32, tag="rr_bc", name="rr_bc", bufs=2)
                    nc.gpsimd.partition_broadcast(rr_bc[:], rr_inv[0:1, :])
                    nc.vector.tensor_tensor(
                        AVT[zo:zo + Z, c, qh * 512:(qh + 1) * 512],
                        pa[0:Z, :],
                        rr_bc[:],
                        Mult,
                    )

                # Qr = Q_nat * (1/r_col) rowwise
                rc_inv = hd.tile([P, NC], F32, tag="rc_inv", name="rc_inv", bufs=2)
                nc.vector.reciprocal(rc_inv[:], r_col[:])
                Qr = hd.tile([P, NC, Z], BF, tag="Qr", name="Qr", bufs=2)
                for qo in range(NC):
                    nc.vector.tensor_scalar_mul(
                        Qr[:, qo, :], Qn[:, qo, h * Z:(h + 1) * Z],
                        rc_inv[:, qo:qo + 1],
                    )

                # ATQ^T (z x k) = Qr^T-contracted with E
                for kh in range(2):
                    pab = ps.tile([P, N], F32, tag="ps_big", name="pab")
                    pa = pab[:, 0:512]
                    for qo in range(NC):
                        nc.tensor.matmul(
                            pa[0:Z, :],
                            Qr[:, qo, :],
                            E[:, qo, kh * 512:(kh + 1) * 512],
                            start=(qo == 0), stop=(qo == NC - 1),
                        )
                    nc.vector.tensor_copy(
                        ATQT[zo:zo + Z, c, kh * 512:(kh + 1) * 512], pa[0:Z, :])

                mlp1_chunk(2 * h)
                mlp1_chunk(2 * h + 1)

            # ---- stage 4: out = AVc @ Wq + ATQc @ Wk + hid @ Wm ----
            for nos in ([0, 1, 2, 3], [4, 5, 6, 7]):   # 2 rounds of 4 n-chunks
                pouts = []
                for i in range(len(nos)):
                    po = ps.tile([P, N], F32, tag="ps_big", name="po")
                    pouts.append(po)
                for ho in range(HC):
                    wmc = stream.tile([P, D], BF, tag="wmc", name="wmc")
                    nc.sync.dma_start(wmc[:], wm_v[:, ho, :])
                    hc = stream.tile([P, N], BF, tag="hc", name="hc")
                    nc.sync.dma_start(hc[:], hid_dram[ho])
                    for i, no in enumerate(nos):
                        nc.tensor.matmul(
                            pouts[i][:, 0:512],
                            hc[:, no * P:(no + 1) * P],
                            wmc[:, 0:512],
                            start=(ho == 0), stop=False,
                        )
                        nc.tensor.matmul(
                            pouts[i][:, 512:768],
                            hc[:, no * P:(no + 1) * P],
                            wmc[:, 512:768],
                            start=(ho == 0), stop=False,
                        )
                for i, no in enumerate(nos):
                    for c2 in range(DC):
                        for lhs, w_sb in ((AVT, wq), (ATQT, wk)):
                            last = (c2 == DC - 1 and lhs is ATQT)
                            nc.tensor.matmul(
                                pouts[i][:, 0:512],
                                lhs[:, c2, no * P:(no + 1) * P],
                                w_sb[:, c2, 0:512],
                                start=False, stop=last,
                            )
                            nc.tensor.matmul(
                                pouts[i][:, 512:768],
                                lhs[:, c2, no * P:(no + 1) * P],
                                w_sb[:, c2, 512:768],
                                start=False, stop=last,
                            )
                for i, no in enumerate(nos):
                    osb = stream.tile([P, D], F32, tag="osb", name="osb", bufs=2)
                    nc.vector.tensor_copy(osb[:], pouts[i][:, 0:768])
                    nc.sync.dma_start(out_v[:, no, :], osb[:])

    nc.compile()
    return nc


def _prep(x, Wq, Wk, betas, W_mlp):
    x = np.asarray(x, dtype=np.float32)
    Wq = np.asarray(Wq, dtype=np.float32)
    Wk = np.asarray(Wk, dtype=np.float32)
    betas = np.asarray(betas, dtype=np.float32)
    W_mlp = np.asarray(W_mlp, dtype=np.float32)

    wq = np.ascontiguousarray(Wq).astype(NPBF)
    wk = np.ascontiguousarray(Wk).astype(NPBF)
    wqT = np.ascontiguousarray(Wq.T).astype(NPBF)
    wkT = np.ascontiguousarray(Wk.T).astype(NPBF)
    wm = np.ascontiguousarray(W_mlp).astype(NPBF)
    wmT = np.ascontiguousarray(W_mlp.T).astype(NPBF)
    # qscale[p, c] = betas[(c*128+p)//64]
    e_idx = (np.arange(DC)[None, :] * P + np.arange(P)[:, None]) // Z
    qscale = betas[e_idx].astype(np.float32)

    in_maps = []
    for b in range(B):
        xT = np.ascontiguousarray(x[b].T).astype(NPBF)
        in_maps.append({
            "xT": xT, "wqT": wqT, "wkT": wkT, "wq": wq, "wk": wk,
            "wmT": wmT, "wm": wm, "qscale": qscale,
        })
    return in_maps


def kernel(x, Wq, Wk, betas, W_mlp, _trace=False):
    if "nc" not in _CACHE:
        _CACHE["nc"] = _build()
    nc = _CACHE["nc"]
    in_maps = _prep(x, Wq, Wk, betas, W_mlp)
    res = run_bass_kernel_spmd(nc, in_maps, core_ids=list(range(B)), trace=_trace)
    out = np.stack([res.results[b]["out"] for b in range(B)], axis=0)
    _CACHE["last_result"] = res
    return out.astype(np.float32)


# revision 14
# speedup vs baseline: 7896.1163x; 1.0604x over previous
"""KQEnergyBlock Trainium2 Bass kernel.

Math (per batch element b, all derived from the reference):
  Q = x @ Wq^T, K = x @ Wk^T                      (N, D), heads h: slices of 64
  S_h = beta_h * Q_h @ K_h^T                      (N, N)
  A_h = softmax(S_h, axis=-1) = E_h / r_h,  E_h = exp(S_h), r_h = rowsum(E_h)
  T1  = sum_h (A_h @ K_h) @ Wq_r[h]  = AVc  @ Wq   (AVc  = concat_h A_h @ K_h)
  T2  = sum_h (A_h^T @ Q_h) @ Wk_r[h] = ATQc @ Wk  (ATQc = concat_h A_h^T @ Q_h)
  mlp = relu(x @ Wm^T) @ Wm
  out = T1 + T2 + mlp

Sharding: data-parallel over batch B=8 across the 8 NeuronCores, one batch
element per core; weights replicated. No collectives.

On-chip layouts (partition dim first):
  xT   [128, 6, 1024]  bf16   (d = c*128+p, n)
  QT   [128, 6, 1024]  bf16   beta-scaled Q^T  (e = c*128+p, q)
  KT   [128, 6, 1024]  bf16   K^T
  Qn   [128, 8,  768]  bf16   Q natural (q = no*128+p, e), raw
  Kn   [128, 8,  768]  bf16   K natural
  E    [128, 8, 1024]  bf16   exp(S_h)   (q = qo*128+p, k)
  ET   [128, 8, 1024]  bf16   exp(S_h^T) (k = ko*128+p, q)
  AVT  [128, 6, 1024]  bf16   concat_h (A_h @ K_h)^T   (e, q)
  ATQT [128, 6, 1024]  bf16   concat_h (A_h^T @ Q_h)^T (e, k)
  hid spilled to DRAM as [24, 128, 1024] bf16 (hid = ho*128+p, n)
"""

import numpy as np
import ml_dtypes

import concourse.mybir as mybir
import concourse.tile as tile
from concourse import bacc
from concourse.bass_utils import run_bass_kernel_spmd

B, N, D = 8, 1024, 768
H, Z = 12, 64
HID = 3072
P = 128
DC = D // P     # 6
NC = N // P     # 8
HC = HID // P   # 24
BF = mybir.dt.bfloat16
F32 = mybir.dt.float32
Exp = mybir.ActivationFunctionType.Exp
Relu = mybir.ActivationFunctionType.Relu
Copy = mybir.ActivationFunctionType.Copy
Mult = mybir.AluOpType.mult

NPBF = ml_dtypes.bfloat16

_CACHE = {}


def _build():
    nc = bacc.Bacc("TRN2", target_bir_lowering=False, debug=False, num_devices=8)
    xT_d = nc.dram_tensor("xT", [D, N], BF, kind="ExternalInput")
    wqT_d = nc.dram_tensor("wqT", [D, D], BF, kind="ExternalInput")
    wkT_d = nc.dram_tensor("wkT", [D, D], BF, kind="ExternalInput")
    wq_d = nc.dram_tensor("wq", [D, D], BF, kind="ExternalInput")
    wk_d = nc.dram_tensor("wk", [D, D], BF, kind="ExternalInput")
    wmT_d = nc.dram_tensor("wmT", [D, HID], BF, kind="ExternalInput")
    wm_d = nc.dram_tensor("wm", [HID, D], BF, kind="ExternalInput")
    qscale_d = nc.dram_tensor("qscale", [P, DC], F32, kind="ExternalInput")
    out_d = nc.dram_tensor("out", [N, D], F32, kind="ExternalOutput")

    # DRAM views with the partition dim innermost-of-row-chunk
    xT_v = xT_d.ap().rearrange("(c p) n -> p c n", p=P)      # [128, 6, 1024]
    wqT_v = wqT_d.ap().rearrange("(c p) e -> p c e", p=P)    # [128, 6, 768]
    wkT_v = wkT_d.ap().rearrange("(c p) e -> p c e", p=P)
    wq_v = wq_d.ap().rearrange("(c p) d -> p c d", p=P)
    wk_v = wk_d.ap().rearrange("(c p) d -> p c d", p=P)
    wmT_v = wmT_d.ap().rearrange("(c p) h -> p c h", p=P)    # [128, 6, 3072]
    wm_v = wm_d.ap().rearrange("(c p) d -> p c d", p=P)      # [128, 24, 768]
    out_v = out_d.ap().rearrange("(c p) d -> p c d", p=P)    # [128, 8, 768]

    with tile.TileContext(nc) as tc:
        with (
            tc.tile_pool(name="acts", bufs=1) as acts,
            tc.tile_pool(name="hd", bufs=1) as hd,
            tc.tile_pool(name="stream", bufs=3) as stream,
            tc.tile_pool(name="ps", bufs=4, space="PSUM") as ps,
            tc.tile_pool(name="dram", bufs=1, space="DRAM") as dram,
        ):
            # ---- input loads ----
            xT = acts.tile([P, DC, N], BF)
            wqT = acts.tile([P, DC, D], BF)
            wkT = acts.tile([P, DC, D], BF)
            wq = acts.tile([P, DC, D], BF)
            wk = acts.tile([P, DC, D], BF)
            qscale = acts.tile([P, DC], F32)
            nc.sync.dma_start(xT[:], xT_v)
            nc.sync.dma_start(wqT[:], wqT_v)
            nc.sync.dma_start(wkT[:], wkT_v)
            nc.sync.dma_start(wq[:], wq_v)
            nc.sync.dma_start(wk[:], wk_v)
            nc.sync.dma_start(qscale[:], qscale_d.ap())

            QT = acts.tile([P, DC, N], BF)
            KT = acts.tile([P, DC, N], BF)
            Qn = acts.tile([P, NC, D], BF)
            Kn = acts.tile([P, NC, D], BF)
            AVT = acts.tile([P, DC, N], BF)
            ATQT = acts.tile([P, DC, N], BF)
            hid_dram = dram.tile([HC, P, N], BF)

            # ---- stage 1: projections ----
            # QT/KT (feature-major):  psum[e_chunk, n] += wT[:, do, e_chunk].T @ xT[:, do, nh]
            for wT_sb, dst, scaled in ((wqT, QT, True), (wkT, KT, False)):
                for eo in range(DC):
                    pt = ps.tile([P, N], F32, tag="ps_big", name="pt")
                    for do in range(DC):
                        for nh in range(2):
                            nc.tensor.matmul(
                                pt[:, nh * 512:(nh + 1) * 512],
                                wT_sb[:, do, eo * P:(eo + 1) * P],
                                xT[:, do, nh * 512:(nh + 1) * 512],
                                start=(do == 0), stop=(do == DC - 1),
                            )
                    if scaled:
                        nc.vector.tensor_scalar_mul(dst[:, eo, :], pt[:],
                                                    qscale[:, eo:eo + 1])
                    else:
                        nc.vector.tensor_copy(dst[:, eo, :], pt[:])
            # Qn/Kn (natural): psum[n_chunk, e] += xT[:, do, n_chunk].T @ wT[:, do, eslice]
            for wT_sb, dst in ((wqT, Qn), (wkT, Kn)):
                for no in range(NC):
                    pt = ps.tile([P, N], F32, tag="ps_big", name="pt")
                    for do in range(DC):
                        nc.tensor.matmul(
                            pt[:, 0:512],
                            xT[:, do, no * P:(no + 1) * P],
                            wT_sb[:, do, 0:512],
                            start=(do == 0), stop=(do == DC - 1),
                        )
                        nc.tensor.matmul(
                            pt[:, 512:768],
                            xT[:, do, no * P:(no + 1) * P],
                            wT_sb[:, do, 512:768],
                            start=(do == 0), stop=(do == DC - 1),
                        )
                    nc.vector.tensor_copy(dst[:, no, :], pt[:, 0:768])

            # ---- stage 2: MLP layer 1 (hid spilled to DRAM) ----
            # emitted interleaved with the head loop below to fill PE idle
            # while ACT is busy with exp
            def mlp1_chunk(ho):
                wt = stream.tile([P, DC, P], BF, tag="wmT", name="wt")
                nc.sync.dma_start(wt[:], wmT_v[:, :, ho * P:(ho + 1) * P])
                pt = ps.tile([P, N], F32, tag="ps_big", name="pt")
                for do in range(DC):
                    for nh in range(2):
                        nc.tensor.matmul(
                            pt[:, nh * 512:(nh + 1) * 512],
                            wt[:, do, :],
                            xT[:, do, nh * 512:(nh + 1) * 512],
                            start=(do == 0), stop=(do == DC - 1),
                        )
                hchunk = stream.tile([P, N], BF, tag="hchunk", name="hchunk")
                nc.vector.tensor_scalar_max(hchunk[:], pt[:], 0.0)
                nc.sync.dma_start(hid_dram[ho], hchunk[:])

            # ---- stage 3: per-head attention ----
            for h in range(H):
                zo = (h % 2) * Z
                c = h // 2
                QT_h = QT[zo:zo + Z, c, :]   # (64, 1024) z x q, beta-scaled
                KT_h = KT[zo:zo + Z, c, :]   # (64, 1024) z x k

                # S (q x k) -> E = exp(S), rowsum in r_col
                E = hd.tile([P, NC, N], BF, tag="E", name="E", bufs=2)
                r_col = hd.tile([P, NC], F32, tag="r_col", name="r_col", bufs=2)
                for qo in range(NC):
                    pt = ps.tile([P, N], F32, tag="ps_big", name="pt")
                    for kh in range(2):
                        nc.tensor.matmul(
                            pt[:, kh * 512:(kh + 1) * 512],
                            QT_h[:, qo * P:(qo + 1) * P],
                            KT_h[:, kh * 512:(kh + 1) * 512],
                            start=True, stop=True,
                        )
                    nc.scalar.activation(E[:, qo, :], pt[:], Exp,
                                         accum_out=r_col[:, qo:qo + 1])
                mlp1_chunk(2 * h)

                # S^T (k x q) -> ET
                ET = hd.tile([P, NC, N], BF, tag="ET", name="ET")
                for ko in range(NC):
                    pt = ps.tile([P, N], F32, tag="ps_big", name="pt")
                    for qh in range(2):
                        nc.tensor.matmul(
                            pt[:, qh * 512:(qh + 1) * 512],
                            KT_h[:, ko * P:(ko + 1) * P],
                            QT_h[:, qh * 512:(qh + 1) * 512],
                            start=True, stop=True,
                        )
                    nc.scalar.activation(ET[:, ko, :], pt[:], Exp)

                # K natural slice for this head + ones column (for rowsum row)
                Kn1 = hd.tile([P, NC, Z + 1], BF, tag="Kn1", name="Kn1", bufs=2)
                for ko in range(NC):
                    nc.vector.tensor_copy(Kn1[:, ko, 0:Z], Kn[:, ko, h * Z:(h + 1) * Z])
                nc.vector.memset(Kn1[:, :, Z:Z + 1], 1.0)

                # Qr = Q_nat * (1/r_col) rowwise
                rc_inv = hd.tile([P, NC], F32, tag="rc_inv", name="rc_inv", bufs=2)
                nc.vector.reciprocal(rc_inv[:], r_col[:])
                Qr = hd.tile([P, NC, Z], BF, tag="Qr", name="Qr", bufs=2)
                for qo in range(NC):
                    nc.vector.tensor_scalar_mul(
                        Qr[:, qo, :], Qn[:, qo, h * Z:(h + 1) * Z],
                        rc_inv[:, qo:qo + 1],
                    )

                # ATQ^T (z x k) = Qr^T-contracted with E
                for kh in range(2):
                    pab = ps.tile([P, N], F32, tag="ps_big", name="pab")
                    pa = pab[:, 0:512]
                    for qo in range(NC):
                        nc.tensor.matmul(
                            pa[0:Z, :],
                            Qr[:, qo, :],
                            E[:, qo, kh * 512:(kh + 1) * 512],
                            start=(qo == 0), stop=(qo == NC - 1),
                        )
                    nc.vector.tensor_copy(
                        ATQT[zo:zo + Z, c, kh * 512:(kh + 1) * 512], pa[0:Z, :])

                # AV^T (z x q) with appended rowsum row; normalize by 1/r_row
                for qh in range(2):
                    rr_inv = hd.tile([1, 512], F32, tag="rr_inv", name="rr_inv", bufs=2)
                    pab = ps.tile([P, N], F32, tag="ps_big", name="pab")
                    pa = pab[:, 0:512]
                    for ko in range(NC):
                        nc.tensor.matmul(
                            pa[0:Z + 1, :],
                            Kn1[:, ko, :],
                            ET[:, ko, qh * 512:(qh + 1) * 512],
                            start=(ko == 0), stop=(ko == NC - 1),
                        )
                    nc.vector.reciprocal(rr_inv[:], pa[Z:Z + 1, :])
                    rr_bc = hd.tile([Z, 512], F32, tag="rr_bc", name="rr_bc", bufs=2)
                    nc.gpsimd.partition_broadcast(rr_bc[:], rr_inv[0:1, :])
                    nc.vector.tensor_tensor(
                        AVT[zo:zo + Z, c, qh * 512:(qh + 1) * 512],
                        pa[0:Z, :],
                        rr_bc[:],
                        Mult,
                    )

                mlp1_chunk(2 * h + 1)

            # ---- stage 4: out = AVc @ Wq + ATQc @ Wk + hid @ Wm ----
            for nos in ([0, 1, 2, 3], [4, 5, 6, 7]):   # 2 rounds of 4 n-chunks
                pouts = []
                for i in range(len(nos)):
                    po = ps.tile([P, N], F32, tag="ps_big", name="po")
                    pouts.append(po)
                for ho in range(HC):
                    wmc = stream.tile([P, D], BF, tag="wmc", name="wmc")
                    nc.sync.dma_start(wmc[:], wm_v[:, ho, :])
                    hc = stream.tile([P, N], BF, tag="hc", name="hc")
                    nc.sync.dma_start(hc[:], hid_dram[ho])
                    for i, no in enumerate(nos):
                        nc.tensor.matmul(
                            pouts[i][:, 0:512],
                            hc[:, no * P:(no + 1) * P],
                            wmc[:, 0:512],
                            start=(ho == 0), stop=False,
                        )
                        nc.tensor.matmul(
                            pouts[i][:, 512:768],
                            hc[:, no * P:(no + 1) * P],
                            wmc[:, 512:768],
                            start=(ho == 0), stop=False,
                        )
                for i, no in enumerate(nos):
                    for c2 in range(DC):
                        for lhs, w_sb in ((AVT, wq), (ATQT, wk)):
                            last = (c2 == DC - 1 and lhs is ATQT)
                            nc.tensor.matmul(
                                pouts[i][:, 0:512],
                                lhs[:, c2, no * P:(no + 1) * P],
                                w_sb[:, c2, 0:512],
                                start=False, stop=last,
                            )
                            nc.tensor.matmul(
                                pouts[i][:, 512:768],
                                lhs[:, c2, no * P:(no + 1) * P],
                                w_sb[:, c2, 512:768],
                                start=False, stop=last,
                            )
                for i, no in enumerate(nos):
                    osb = stream.tile([P, D], F32, tag="osb", name="osb", bufs=2)
                    nc.vector.tensor_copy(osb[:], pouts[i][:, 0:768])
                    nc.sync.dma_start(out_v[:, no, :], osb[:])

    nc.compile()
    return nc


def _prep(x, Wq, Wk, betas, W_mlp):
    x = np.asarray(x, dtype=np.float32)
    Wq = np.asarray(Wq, dtype=np.float32)
    Wk = np.asarray(Wk, dtype=np.float32)
    betas = np.asarray(betas, dtype=np.float32)
    W_mlp = np.asarray(W_mlp, dtype=np.float32)

    wq = np.ascontiguousarray(Wq).astype(NPBF)
    wk = np.ascontiguousarray(Wk).astype(NPBF)
    wqT = np.ascontiguousarray(Wq.T).astype(NPBF)
    wkT = np.ascontiguousarray(Wk.T).astype(NPBF)
    wm = np.ascontiguousarray(W_mlp).astype(NPBF)
    wmT = np.ascontiguousarray(W_mlp.T).astype(NPBF)
    # qscale[p, c] = betas[(c*128+p)//64]
    e_idx = (np.arange(DC)[None, :] * P + np.arange(P)[:, None]) // Z
    qscale = betas[e_idx].astype(np.float32)

    in_maps = []
    for b in range(B):
        xT = np.ascontiguousarray(x[b].T).astype(NPBF)
        in_maps.append({
            "xT": xT, "wqT": wqT, "wkT": wkT, "wq": wq, "wk": wk,
            "wmT": wmT, "wm": wm, "qscale": qscale,
        })
    return in_maps


def kernel(x, Wq, Wk, betas, W_mlp, _trace=False):
    if "nc" not in _CACHE:
        _CACHE["nc"] = _build()
    nc = _CACHE["nc"]
    in_maps = _prep(x, Wq, Wk, betas, W_mlp)
    res = run_bass_kernel_spmd(nc, in_maps, core_ids=list(range(B)), trace=_trace)
    out = np.stack([res.results[b]["out"] for b in range(B)], axis=0)
    _CACHE["last_result"] = res
    return out.astype(np.float32)
